# revision 30
# baseline (speedup 1.0000x reference)
"""Trainium2 Bass kernel for the AttentionBlock problem (v3).

Full inputs -> full output; sharded over 8 NeuronCores (core c owns
queries [1024*c, 1024*(c+1))); every core gets the full key-side x, so
no on-device collectives.

Two hardware facts drive the structure (both measured from NTFF traces
on this part):
  1. All matmul dtypes stream 1 column/cycle; the PE clock is 1.2 GHz
     while the Vector engine is active and 2.4 GHz when DVE is quiet
     (shared power cap: DVE activity clamps the PE clock).  So ALL
     DVE work (projection lo-splits, max-pass reduces, normalize) is
     packed into a prefix/tail, and the main score*V loop runs with the
     vector engine silent -> 2.4 GHz matmuls, ~2x faster.
  2. fp32 matmuls run ~2.5x slower (LOW_HIGH) -> everything on the PE
     is bf16, with hi/lo splits for accuracy.

Math (per core, N=8192 keys, Nq=1024 queries, d=64):
  projections (bf16 hi/lo, x pre-split on host):
    Q = x@R/8 = [Rh;Rl]^T [xh;xh] + Rh^T xl     (2 MMs / 512 cols)
  pass-1 (max estimate), 2x row-group packed: row groups 0-1 compute
    K_hi.Q_hi for keys [k, k+512), groups 2-3 for [k+512, k+1024)
    concurrently (contraction is only 64) -> half the PE passes.
    Per-query m: even row-tiles DVE reduce_max; odd row-tiles ACT
    log-sum-exp (T=10, exp accum_out; m = T*ln(sum) in [max, max+28])
    with ln computed from the fp32 exponent bits (no Ln table load).
  scores, ONE fp16 MM per 128-key block into PSUM (fp16 streams at
  full rate like bf16 on this part):
    [K_f16(64); K_lo_f16(dims 0-62); 1]^T [Q_f16(64); Q_f16(0-62); -m]
  i.e. K carried to ~22 bits on 63 dims, Q at 11 bits; the host permutes
  the R/E columns so the weakest |E_j||R_j| product sits at dim 63 where
  the lo term is dropped.  CPU-simmed rel err 1.24e-2 (gate 2e-2).
  P = bf16(exp(S)) - bf16's e^88 range absorbs the lse overshoot
  PV: out_aug^T += xaug_j^T P_j   (xaug = [x_bf16, 1])
  normalize (deferred to tail): out = out_aug[0:64] / out_aug[64]
"""

import numpy as np
from contextlib import ExitStack

import concourse.bass as bass
import concourse.tile as tile
from concourse import bacc, mybir

N = 8192
D = 64
NCORES = 8
NQ = N // NCORES          # 1024 queries per core
NKB = N // 128            # 64 key blocks
QC = 512                  # query chunk (pass-2 free dim)
DP = 72                   # xaug row stride (16B aligned in bf16)
T_LSE = 10.0              # lse temperature for the ACT max-proxy
LN2 = 0.6931471805599453

F32 = mybir.dt.float32
BF16 = mybir.dt.bfloat16
F16 = mybir.dt.float16
I32 = mybir.dt.int32
AX = mybir.AxisListType.X
SUB = mybir.AluOpType.subtract
EXP = mybir.ActivationFunctionType.Exp
COPY = mybir.ActivationFunctionType.Copy


def build():
    nc = bacc.Bacc("TRN2", target_bir_lowering=False, debug=False, num_devices=1)

    # combined small-input tensor: [128, 0:256] = weights (rhl|ehl|rh|eh),
    # [128, 256:1280] = xqhh, [0:64, 1280:2304] = xqlo, [1, 2304:10496] = ones
    wx_ap = nc.dram_tensor("wx", [128, 2304 + N], BF16, kind="ExternalInput").ap()
    xt_ap = nc.dram_tensor("xt", [128, 2 * N], BF16, kind="ExternalInput").ap()
    id_ap = nc.dram_tensor("ident", [128, 128], F32, kind="ExternalInput").ap()
    xaug_ap = nc.dram_tensor("xaug", [128, NKB * DP], BF16,
                             kind="ExternalInput").ap()
    out_ap = nc.dram_tensor("out", [NQ, D], F32, kind="ExternalOutput").ap()

    with tile.TileContext(nc) as tc, ExitStack() as ctx:
        const = ctx.enter_context(tc.tile_pool(name="const", bufs=1))
        big = ctx.enter_context(tc.tile_pool(name="big", bufs=1))
        work = ctx.enter_context(tc.tile_pool(name="work", bufs=3))
        small = ctx.enter_context(tc.tile_pool(name="small", bufs=2))
        # PSUM: ppool 3x[128,1024] = 6 banks, pacc 2x[65,512] = 2 banks
        ppool = ctx.enter_context(tc.tile_pool(name="pp", bufs=3, space="PSUM"))
        pacc = ctx.enter_context(tc.tile_pool(name="pacc", bufs=2, space="PSUM"))

        # ---------------- tiles + input DMA ----------------
        # kt_f [128, N] fp16: rows 0-63 = fp16(K), rows 64-126 =
        # fp16(K - fp16(K)) dims 0-62, row 127 = ones.
        # ktf_hh [128, N] fp16: rows 64-126 = fp16(K) dims 0-62 (the rhs
        # of the row-group-packed pass-1 matmuls).
        kt_f = big.tile([128, N], F16, name="ktf")
        ktf_hh = big.tile([128, N], F16, name="ktfhh")
        # one combined DMA for weights + query-slice inputs + ones row
        wx_sb = big.tile([128, 2304], BF16, name="wx")
        nc.sync.dma_start(wx_sb[:], wx_ap[:, 0:2304])
        nc.sync.dma_start(kt_f[127:128, :].bitcast(BF16),
                          wx_ap[0:1, 2304:2304 + N])
        rhl_sb = wx_sb[:, 0:64]
        ehl_sb = wx_sb[:, 64:128]
        rh_sb = wx_sb[0:D, 128:192]
        eh_sb = wx_sb[0:D, 192:256]
        rh2_sb = wx_sb[D:128, 128:192]    # rh again, partitions 64..127
        eh2_sb = wx_sb[D:128, 192:256]    # eh again, partitions 64..127
        xqhh_sb = wx_sb[:, 256:1280]
        xqlo_sb = wx_sb[:, 1280:2304]
        xqlo2_sb = xqlo_sb
        # key-side x: 8 paired chunk-tiles [128, 2048]: cols 0:1024 = x^T
        # hi (dup to 128 rows), cols 1024:2048 rows 0:63 = x^T lo
        xtc_sb = [big.tile([128, 2048], BF16, name=f"xtc{i}") for i in range(8)]
        for i in range(8):
            nc.sync.dma_start(xtc_sb[i][:], xt_ap[:, i * 2048:(i + 1) * 2048])
        ident = const.tile([128, 128], F32)
        nc.sync.dma_start(ident[:], id_ap[:])
        # xaug pre-packed on host into SBUF layout [128, block*72]
        xaug_sb = big.tile([128, NKB * DP], BF16)
        nc.sync.dma_start(xaug_sb[:], xaug_ap[:])
        xaug_v = xaug_sb[:].rearrange("p (t d) -> p t d", d=DP)[:, :, 0:D + 1]

        qst_f = big.tile([128, NQ], F16)  # Q_f16; dup dims 0-62 at 64-126;

        # ---------------- projections + pass 1, interleaved ------------
        # Pass-1 slots for key-quarter q are emitted right after quarter
        # q's K projections so the ACT/DVE consumers (the prefix
        # bottleneck) start ~35us earlier than a phase-serial order.
        pqs = []
        for s in range(NQ // 512):
            pq_t = ppool.tile([128, 1024], F32, tag="pp", name="pq")
            pqs.append(pq_t[0:D, 0:512])
            nc.tensor.matmul(pqs[s], rhl_sb, xqhh_sb[:, s * 512:(s + 1) * 512],
                             start=True, stop=False)
        nc.tensor.matmul(pqs[0], rh_sb, xqlo_sb[0:D, 0:512],
                         start=False, stop=True)
        nc.tensor.matmul(pqs[1], rh2_sb, xqlo2_sb[D:128, 512:1024],
                         start=False, stop=True, tile_position=(64, 0))
        for s in range(NQ // 512):
            sl = slice(s * 512, (s + 1) * 512)
            pq = pqs[s]
            nc.scalar.copy(qst_f[0:D, sl], pq)
            nc.vector.tensor_copy(qst_f[D:127, sl], qst_f[0:D - 1, sl])

        # pass-1 state: 8 row-tiles of 128 queries; slot (rt, c) covers
        # keys [1024c, 1024(c+1)) with two row-group-packed MMs.
        mx = work.tile([128, 40], F32, tag="mx", name="mx")
        nc.vector.memset(mx[:], 0.0)
        mxp = work.tile([128, 64], F32, tag="mxp", name="mxp")

        def emit_p1_slot(rt, c, eng):
            q0 = rt * 128
            mcol = rt * 8 + c
            ps1_t = ppool.tile([128, 1024], F32, tag="pp", name="ps1")
            k0 = c * 1024
            nc.tensor.matmul(ps1_t[:, 0:512],
                             qst_f[0:D, q0:q0 + 128],
                             kt_f[0:D, k0:k0 + 512],
                             start=True, stop=True)
            nc.tensor.matmul(ps1_t[:, 512:1024],
                             qst_f[D:127, q0:q0 + 128],
                             ktf_hh[D:127, k0 + 512:k0 + 1024],
                             start=True, stop=True, tile_position=(64, 0))
            if eng == "dve":
                nc.vector.reduce_max(mxp[:, mcol:mcol + 1], ps1_t[:], axis=AX)
            else:
                scr = work.tile([128, 1024], BF16, tag="lsescr", name="lsescr")
                nc.scalar.activation(scr[:], ps1_t[:], EXP, scale=1.0 / T_LSE,
                                     accum_out=mxp[:, mcol:mcol + 1])

        def finish_p1_rt(rt, eng):
            if eng == "dve":
                nc.vector.reduce_max(mx[:, rt:rt + 1], mxp[:, rt * 8:rt * 8 + 8],
                                     axis=AX, negate=True)
            else:
                # -m = -T*ln(ssum); ln from the fp32 exponent bits
                # (ACT's Ln table is wrong for huge inputs and would
                # thrash the Exp table set anyway):
                # bits(s)*2^-23 ~= log2(s) + 127  (within +0.086)
                ssum = small.tile([128, 1], F32, tag="ssum", name="ssum")
                nc.vector.reduce_sum(ssum[:], mxp[:, rt * 8:rt * 8 + 8], axis=AX)
                ibits = small.tile([128, 1], F32, tag="ibits", name="ibits")
                nc.vector.tensor_copy(ibits[:], ssum[:].bitcast(I32))
                nc.scalar.activation(mx[:, rt:rt + 1], ibits[:], COPY,
                                     scale=-T_LSE * LN2 * 2.0 ** -23,
                                     bias=127.0 * T_LSE * LN2)

        # K projections: chunk pairs with the hl/hl then lo/lo order so
        # adjacent matmuls hit different PSUM banks (same-bank accumulate
        # pairs serialize the PE fill/drain) and reuse the same weights.
        # even/odd DVE/lse split: DVE's queue drains ~10us before ACT's
        # lse tail, and the PE clock un-throttles as soon as DVE goes
        # quiet - ACT being the pass-1 pacer costs nothing.  (Measured:
        # shifting slots either direction is slower - more DVE delays
        # the un-throttle, fewer DVE lengthens the ACT tail.)
        ENG = {rt: ("dve" if rt % 2 == 0 else "lse") for rt in range(8)}
        # K-projection pair p produces exactly the keys pass-1 c-group p
        # consumes, so the two are fused: the PE-paced projections hide
        # under the ACT/DVE-paced pass-1 consumers instead of running as
        # a serial phase before them.
        for p in range(8):
            s0 = 2 * p
            pks = []
            for s in (s0, s0 + 1):
                pk_t = ppool.tile([128, 1024], F32, tag="pp", name="pk")
                pks.append(pk_t[0:D, 0:512])
                xi, xo = divmod(s, 2)
                nc.tensor.matmul(pks[-1], ehl_sb,
                                 xtc_sb[xi][:, xo * 512:(xo + 1) * 512],
                                 start=True, stop=False)
            xi0, xo0 = divmod(s0, 2)
            nc.tensor.matmul(pks[0], eh_sb,
                             xtc_sb[xi0][0:D, 1024 + xo0 * 512:1024 + (xo0 + 1) * 512],
                             start=False, stop=True)
            xi1, xo1 = divmod(s0 + 1, 2)
            nc.tensor.matmul(pks[1], eh2_sb,
                             xtc_sb[xi1][D:128, 1024 + xo1 * 512:1024 + (xo1 + 1) * 512],
                             start=False, stop=True, tile_position=(64, 0))
            for i, s in enumerate((s0, s0 + 1)):
                sl = slice(s * 512, (s + 1) * 512)
                nc.scalar.copy(kt_f[0:D, sl], pks[i])
                nc.vector.tensor_copy(ktf_hh[D:127, sl], kt_f[0:D - 1, sl])
                nc.vector.tensor_tensor(out=kt_f[D:127, sl], in0=pks[i][0:D - 1, :],
                                        in1=kt_f[0:D - 1, sl], op=SUB)
            for rt in range(8):
                emit_p1_slot(rt, p, ENG[rt])
        for rt in range(8):
            finish_p1_rt(rt, ENG[rt])

        # single transpose: mx cols 0-7 land on partitions 0-7 of ps_m;
        # one DVE cast then one partition->free gather DMA into row 127
        pm_t = ppool.tile([128, 1024], F32, tag="pp", name="pm")
        ps_m = pm_t[0:32, 0:128]
        nc.tensor.transpose(ps_m, mx[:, 0:32], ident[:])
        stag = small.tile([8, 128], F16, tag="srow", name="stag")
        nc.vector.tensor_copy(stag[:], ps_m[0:8, :])
        nc.sync.dma_start(
            qst_f[127:128, :].rearrange("p (t c) -> p t c", c=128),
            stag[:].rearrange("t c -> t () c"))

        # ---------------- main loop (vector engine silent) -------------
        def emit_score(qc, jj):
            pexp_t = ppool.tile([128, 1024], F32, tag="pp", name="pexp")
            qsl = slice(qc * QC, (qc + 1) * QC)
            for h in range(2):
                j = jj + h
                blk = slice(j * 128, (j + 1) * 128)
                reg = pexp_t[:, h * 512:(h + 1) * 512]
                nc.tensor.matmul(reg, kt_f[:, blk], qst_f[:, qsl],
                                 start=True, stop=True)
            return pexp_t

        po = {}
        pexp_cur = emit_score(0, 0)
        for u in range(64):
            qc, jj = u // 32, 2 * (u % 32)
            if jj == 0:
                po[qc] = pacc.tile([D + 1, QC], F32, tag="po", name="po")
            pt = work.tile([128, 1024], BF16, tag="pt", name="pt")
            nc.scalar.activation(pt[:], pexp_cur[:], EXP)
            if u + 1 < 64:
                pexp_cur = emit_score((u + 1) // 32, 2 * ((u + 1) % 32))
            nc.tensor.matmul(po[qc][:], xaug_v[:, jj, :], pt[:, 0:512],
                             start=(jj == 0), stop=False)
            nc.tensor.matmul(po[qc][:], xaug_v[:, jj + 1, :], pt[:, 512:1024],
                             start=False, stop=(jj == 62))

        # ---------------- normalize (tail) ----------------
        o_all = big.tile([128, NQ // 128 * D], F32, name="o_all")
        ots = []
        for qc in range(2):
            ot = work.tile([D + 1, QC], F32, tag="ot", name="ot")
            nc.vector.tensor_copy(ot[:], po[qc][:])
            ots.append(ot)
        # interleave the two chunks' normalize chains so DVE recip/mul
        # pipeline under the other chunk's PE transposes
        for h in range(QC // 128):
            for qc in range(2):
                ptr_t = ppool.tile([128, 1024], F32, tag="pp", name="ptr")
                ps_t = ptr_t[0:128, 0:D + 1]
                nc.tensor.transpose(ps_t, ots[qc][:, h * 128:(h + 1) * 128],
                                    ident[0:D + 1, 0:D + 1])
                recip = small.tile([128, 1], F32, tag="recip", name="recip")
                nc.vector.reciprocal(recip[:], ps_t[:, D:D + 1])
                t = qc * 4 + h
                nc.vector.tensor_scalar_mul(o_all[:, t * D:(t + 1) * D],
                                            ps_t[:, 0:D], recip[:])
        nc.sync.dma_start(
            out_ap.rearrange("(t p) d -> p t d", p=128),
            o_all[:].rearrange("p (t d) -> p t d", d=D))

    nc.compile()
    return nc


_CACHE = {}


def _get_nc():
    if "nc" not in _CACHE:
        _CACHE["nc"] = build()
    return _CACHE["nc"]


def kernel(x, rotation_params, entangle_params, _trace=False, _nc=None):
    from concourse.bass_utils import run_bass_kernel_spmd
    import ml_dtypes

    bf16 = ml_dtypes.bfloat16
    f32 = np.float32

    x = np.ascontiguousarray(x, dtype=f32)
    rs = np.ascontiguousarray(rotation_params, dtype=f32) / 8.0
    e = np.ascontiguousarray(entangle_params, dtype=f32)
    # permute projection columns so the weakest |E_j|*|R_j| product sits
    # at dim 63, whose fp16 lo-correction is the one dropped
    perm = np.argsort(-(np.linalg.norm(e, axis=0) * np.linalg.norm(rs, axis=0)))
    rs = np.ascontiguousarray(rs[:, perm])
    e = np.ascontiguousarray(e[:, perm])

    xh = x.astype(bf16)
    xl = (x - xh.astype(f32)).astype(bf16)
    xthh = np.ascontiguousarray(np.vstack([xh.T, xh.T]))          # [128, N]
    xtlo = xl.T                                                   # [64, N]
    # paired key-side tiles: [128, 2048] per 1024 cols: hi | lo
    xt = np.zeros((128, 2 * N), dtype=bf16)
    for i in range(8):
        xt[:, i * 2048:i * 2048 + 1024] = xthh[:, i * 1024:(i + 1) * 1024]
        xt[0:D, i * 2048 + 1024:(i + 1) * 2048] = xtlo[:, i * 1024:(i + 1) * 1024]
        xt[D:128, i * 2048 + 1024:(i + 1) * 2048] = xtlo[:, i * 1024:(i + 1) * 1024]

    def hl(w):
        h = w.astype(bf16)
        l = (w - h.astype(f32)).astype(bf16)
        return np.ascontiguousarray(np.vstack([h, l])), h

    rhl, rh = hl(rs)
    ehl, eh = hl(e)

    xaug = np.zeros((N, DP), dtype=bf16)
    xaug[:, :D] = xh
    xaug[:, D] = 1.0
    xaug_p = np.ascontiguousarray(
        xaug.reshape(NKB, 128, DP).transpose(1, 0, 2).reshape(128, NKB * DP))

    ident = np.eye(128, dtype=f32)

    nc = _nc if _nc is not None else _get_nc()
    in_maps = []
    for c in range(NCORES):
        qsl = slice(c * NQ, (c + 1) * NQ)
        wx = np.zeros((128, 2304 + N), dtype=bf16)
        wx[:, 0:64] = rhl
        wx[:, 64:128] = ehl
        wx[0:D, 128:192] = rh
        wx[0:D, 192:256] = eh
        wx[D:128, 128:192] = rh
        wx[D:128, 192:256] = eh
        wx[:, 256:1280] = xthh[:, qsl]
        wx[0:D, 1280:2304] = xtlo[:, qsl]
        wx[D:128, 1280:2304] = xtlo[:, qsl]
        # kt_f's ones row is fp16; store fp16(1.0)'s bit pattern (0x3C00)
        # through the bf16-typed wx buffer
        wx.view(np.uint16)[0:1, 2304:] = 0x3C00
        in_maps.append({
            "wx": wx,
            "xt": xt,
            "ident": ident,
            "xaug": xaug_p,
        })
    res = run_bass_kernel_spmd(nc, in_maps, core_ids=list(range(NCORES)),
                               trace=_trace)
    out = np.concatenate([res.results[c]["out"] for c in range(NCORES)], axis=0)
    if _trace:
        return out, res
    return out


# revision 31
# speedup vs baseline: 1.0028x; 1.0028x over previous
"""Trainium2 Bass kernel for the AttentionBlock problem (v3).

Full inputs -> full output; sharded over 8 NeuronCores (core c owns
queries [1024*c, 1024*(c+1))); every core gets the full key-side x, so
no on-device collectives.

Two hardware facts drive the structure (both measured from NTFF traces
on this part):
  1. All matmul dtypes stream 1 column/cycle; the PE clock is 1.2 GHz
     while the Vector engine is active and 2.4 GHz when DVE is quiet
     (shared power cap: DVE activity clamps the PE clock).  So ALL
     DVE work (projection lo-splits, max-pass reduces, normalize) is
     packed into a prefix/tail, and the main score*V loop runs with the
     vector engine silent -> 2.4 GHz matmuls, ~2x faster.
  2. fp32 matmuls run ~2.5x slower (LOW_HIGH) -> everything on the PE
     is bf16, with hi/lo splits for accuracy.

Math (per core, N=8192 keys, Nq=1024 queries, d=64):
  projections (bf16 hi/lo, x pre-split on host):
    Q = x@R/8 = [Rh;Rl]^T [xh;xh] + Rh^T xl     (2 MMs / 512 cols)
  pass-1 (max estimate), 2x row-group packed: row groups 0-1 compute
    K_hi.Q_hi for keys [k, k+512), groups 2-3 for [k+512, k+1024)
    concurrently (contraction is only 64) -> half the PE passes.
    Per-query m: even row-tiles DVE reduce_max; odd row-tiles ACT
    log-sum-exp (T=10, exp accum_out; m = T*ln(sum) in [max, max+28])
    with ln computed from the fp32 exponent bits (no Ln table load).
  scores, ONE fp16 MM per 128-key block into PSUM (fp16 streams at
  full rate like bf16 on this part):
    [K_f16(64); K_lo_f16(dims 0-62); 1]^T [Q_f16(64); Q_f16(0-62); -m]
  i.e. K carried to ~22 bits on 63 dims, Q at 11 bits; the host permutes
  the R/E columns so the weakest |E_j||R_j| product sits at dim 63 where
  the lo term is dropped.  CPU-simmed rel err 1.24e-2 (gate 2e-2).
  P = bf16(exp(S)) - bf16's e^88 range absorbs the lse overshoot
  PV: out_aug^T += xaug_j^T P_j   (xaug = [x_bf16, 1])
  normalize (deferred to tail): out = out_aug[0:64] / out_aug[64]
"""

import numpy as np
from contextlib import ExitStack

import concourse.bass as bass
import concourse.tile as tile
from concourse import bacc, mybir

N = 8192
D = 64
NCORES = 8
NQ = N // NCORES          # 1024 queries per core
NKB = N // 128            # 64 key blocks
QC = 512                  # query chunk (pass-2 free dim)
DP = 72                   # xaug row stride (16B aligned in bf16)
T_LSE = 10.0              # lse temperature for the ACT max-proxy
LN2 = 0.6931471805599453

F32 = mybir.dt.float32
BF16 = mybir.dt.bfloat16
F16 = mybir.dt.float16
I32 = mybir.dt.int32
AX = mybir.AxisListType.X
SUB = mybir.AluOpType.subtract
EXP = mybir.ActivationFunctionType.Exp
COPY = mybir.ActivationFunctionType.Copy


def build():
    nc = bacc.Bacc("TRN2", target_bir_lowering=False, debug=False, num_devices=1)

    # combined small-input tensor: [128, 0:256] = weights (rhl|ehl|rh|eh),
    # [128, 256:1280] = xqhh, [0:64, 1280:2304] = xqlo, [1, 2304:10496] = ones
    wx_ap = nc.dram_tensor("wx", [128, 2304 + N], BF16, kind="ExternalInput").ap()
    xt_ap = nc.dram_tensor("xt", [128, 2 * N], BF16, kind="ExternalInput").ap()
    id_ap = nc.dram_tensor("ident", [128, 128], F32, kind="ExternalInput").ap()
    xaug_ap = nc.dram_tensor("xaug", [128, NKB * DP], BF16,
                             kind="ExternalInput").ap()
    out_ap = nc.dram_tensor("out", [NQ, D], F32, kind="ExternalOutput").ap()

    with tile.TileContext(nc) as tc, ExitStack() as ctx:
        const = ctx.enter_context(tc.tile_pool(name="const", bufs=1))
        big = ctx.enter_context(tc.tile_pool(name="big", bufs=1))
        work = ctx.enter_context(tc.tile_pool(name="work", bufs=3))
        small = ctx.enter_context(tc.tile_pool(name="small", bufs=2))
        # PSUM: ppool 3x[128,1024] = 6 banks, pacc 2x[65,512] = 2 banks
        ppool = ctx.enter_context(tc.tile_pool(name="pp", bufs=3, space="PSUM"))
        pacc = ctx.enter_context(tc.tile_pool(name="pacc", bufs=2, space="PSUM"))

        # ---------------- tiles + input DMA ----------------
        # kt_f [128, N] fp16: rows 0-63 = fp16(K), rows 64-126 =
        # fp16(K - fp16(K)) dims 0-62, row 127 = ones.
        # ktf_hh [128, N] fp16: rows 64-126 = fp16(K) dims 0-62 (the rhs
        # of the row-group-packed pass-1 matmuls).
        kt_f = big.tile([128, N], F16, name="ktf")
        ktf_hh = big.tile([128, N], F16, name="ktfhh")
        # one combined DMA for weights + query-slice inputs + ones row
        wx_sb = big.tile([128, 2304], BF16, name="wx")
        nc.sync.dma_start(wx_sb[:], wx_ap[:, 0:2304])
        nc.sync.dma_start(kt_f[127:128, :].bitcast(BF16),
                          wx_ap[0:1, 2304:2304 + N])
        rhl_sb = wx_sb[:, 0:64]
        ehl_sb = wx_sb[:, 64:128]
        rh_sb = wx_sb[0:D, 128:192]
        eh_sb = wx_sb[0:D, 192:256]
        rh2_sb = wx_sb[D:128, 128:192]    # rh again, partitions 64..127
        eh2_sb = wx_sb[D:128, 192:256]    # eh again, partitions 64..127
        xqhh_sb = wx_sb[:, 256:1280]
        xqlo_sb = wx_sb[:, 1280:2304]
        xqlo2_sb = xqlo_sb
        # key-side x: 8 paired chunk-tiles [128, 2048]: cols 0:1024 = x^T
        # hi (dup to 128 rows), cols 1024:2048 rows 0:63 = x^T lo
        xtc_sb = [big.tile([128, 2048], BF16, name=f"xtc{i}") for i in range(8)]
        for i in range(8):
            nc.sync.dma_start(xtc_sb[i][:], xt_ap[:, i * 2048:(i + 1) * 2048])
        ident = const.tile([128, 128], F32)
        nc.sync.dma_start(ident[:], id_ap[:])
        # xaug pre-packed on host into SBUF layout [128, block*72]
        xaug_sb = big.tile([128, NKB * DP], BF16)
        nc.sync.dma_start(xaug_sb[:], xaug_ap[:])
        xaug_v = xaug_sb[:].rearrange("p (t d) -> p t d", d=DP)[:, :, 0:D + 1]

        qst_f = big.tile([128, NQ], F16)  # Q_f16; dup dims 0-62 at 64-126;

        # ---------------- projections + pass 1, interleaved ------------
        # Pass-1 slots for key-quarter q are emitted right after quarter
        # q's K projections so the ACT/DVE consumers (the prefix
        # bottleneck) start ~35us earlier than a phase-serial order.
        pqs = []
        for s in range(NQ // 512):
            pq_t = ppool.tile([128, 1024], F32, tag="pp", name="pq")
            pqs.append(pq_t[0:D, 0:512])
            nc.tensor.matmul(pqs[s], rhl_sb, xqhh_sb[:, s * 512:(s + 1) * 512],
                             start=True, stop=False)
        nc.tensor.matmul(pqs[0], rh_sb, xqlo_sb[0:D, 0:512],
                         start=False, stop=True)
        nc.tensor.matmul(pqs[1], rh2_sb, xqlo2_sb[D:128, 512:1024],
                         start=False, stop=True, tile_position=(64, 0))
        for s in range(NQ // 512):
            sl = slice(s * 512, (s + 1) * 512)
            pq = pqs[s]
            nc.scalar.copy(qst_f[0:D, sl], pq)
            nc.vector.tensor_copy(qst_f[D:127, sl], qst_f[0:D - 1, sl])

        # pass-1 state: 8 row-tiles of 128 queries; slot (rt, c) covers
        # keys [1024c, 1024(c+1)) with two row-group-packed MMs.
        mx = work.tile([128, 40], F32, tag="mx", name="mx")
        nc.vector.memset(mx[:], 0.0)
        mxp = work.tile([128, 64], F32, tag="mxp", name="mxp")

        def emit_p1_slot(rt, c, eng):
            q0 = rt * 128
            mcol = rt * 8 + c
            ps1_t = ppool.tile([128, 1024], F32, tag="pp", name="ps1")
            k0 = c * 1024
            nc.tensor.matmul(ps1_t[:, 0:512],
                             qst_f[0:D, q0:q0 + 128],
                             kt_f[0:D, k0:k0 + 512],
                             start=True, stop=True)
            nc.tensor.matmul(ps1_t[:, 512:1024],
                             qst_f[D:127, q0:q0 + 128],
                             ktf_hh[D:127, k0 + 512:k0 + 1024],
                             start=True, stop=True, tile_position=(64, 0))
            if eng == "dve":
                nc.vector.reduce_max(mxp[:, mcol:mcol + 1], ps1_t[:], axis=AX)
            else:
                scr = work.tile([128, 1024], BF16, tag="lsescr", name="lsescr")
                nc.scalar.activation(scr[:], ps1_t[:], EXP, scale=1.0 / T_LSE,
                                     accum_out=mxp[:, mcol:mcol + 1])

        def finish_p1_rt(rt, eng):
            if eng == "dve":
                nc.vector.reduce_max(mx[:, rt:rt + 1], mxp[:, rt * 8:rt * 8 + 8],
                                     axis=AX, negate=True)
            else:
                # -m = -T*ln(ssum); ln from the fp32 exponent bits
                # (ACT's Ln table is wrong for huge inputs and would
                # thrash the Exp table set anyway):
                # bits(s)*2^-23 ~= log2(s) + 127  (within +0.086)
                ssum = small.tile([128, 1], F32, tag="ssum", name="ssum")
                nc.vector.reduce_sum(ssum[:], mxp[:, rt * 8:rt * 8 + 8], axis=AX)
                ibits = small.tile([128, 1], F32, tag="ibits", name="ibits")
                nc.vector.tensor_copy(ibits[:], ssum[:].bitcast(I32))
                nc.scalar.activation(mx[:, rt:rt + 1], ibits[:], COPY,
                                     scale=-T_LSE * LN2 * 2.0 ** -23,
                                     bias=127.0 * T_LSE * LN2)

        # K projections: chunk pairs with the hl/hl then lo/lo order so
        # adjacent matmuls hit different PSUM banks (same-bank accumulate
        # pairs serialize the PE fill/drain) and reuse the same weights.
        # even/odd DVE/lse split: DVE's queue drains ~10us before ACT's
        # lse tail, and the PE clock un-throttles as soon as DVE goes
        # quiet - ACT being the pass-1 pacer costs nothing.  (Measured:
        # shifting slots either direction is slower - more DVE delays
        # the un-throttle, fewer DVE lengthens the ACT tail.)
        ENG = {rt: ("dve" if rt % 2 == 0 else "lse") for rt in range(8)}
        # K-projection pair p produces exactly the keys pass-1 c-group p
        # consumes, so the two are fused: the PE-paced projections hide
        # under the ACT/DVE-paced pass-1 consumers instead of running as
        # a serial phase before them.
        for p in range(8):
            s0 = 2 * p
            pks = []
            for s in (s0, s0 + 1):
                pk_t = ppool.tile([128, 1024], F32, tag="pp", name="pk")
                pks.append(pk_t[0:D, 0:512])
                xi, xo = divmod(s, 2)
                nc.tensor.matmul(pks[-1], ehl_sb,
                                 xtc_sb[xi][:, xo * 512:(xo + 1) * 512],
                                 start=True, stop=False)
            xi0, xo0 = divmod(s0, 2)
            nc.tensor.matmul(pks[0], eh_sb,
                             xtc_sb[xi0][0:D, 1024 + xo0 * 512:1024 + (xo0 + 1) * 512],
                             start=False, stop=True)
            xi1, xo1 = divmod(s0 + 1, 2)
            nc.tensor.matmul(pks[1], eh2_sb,
                             xtc_sb[xi1][D:128, 1024 + xo1 * 512:1024 + (xo1 + 1) * 512],
                             start=False, stop=True, tile_position=(64, 0))
            for i, s in enumerate((s0, s0 + 1)):
                sl = slice(s * 512, (s + 1) * 512)
                nc.scalar.copy(kt_f[0:D, sl], pks[i])
                nc.vector.tensor_copy(ktf_hh[D:127, sl], kt_f[0:D - 1, sl])
                nc.vector.tensor_tensor(out=kt_f[D:127, sl], in0=pks[i][0:D - 1, :],
                                        in1=kt_f[0:D - 1, sl], op=SUB)
            for rt in range(8):
                emit_p1_slot(rt, p, ENG[rt])
        for rt in range(8):
            finish_p1_rt(rt, ENG[rt])

        # single transpose: mx cols 0-7 land on partitions 0-7 of ps_m;
        # one DVE cast then one partition->free gather DMA into row 127
        pm_t = ppool.tile([128, 1024], F32, tag="pp", name="pm")
        ps_m = pm_t[0:32, 0:128]
        nc.tensor.transpose(ps_m, mx[:, 0:32], ident[:])
        stag = small.tile([8, 128], F16, tag="srow", name="stag")
        nc.vector.tensor_copy(stag[:], ps_m[0:8, :])
        nc.sync.dma_start(
            qst_f[127:128, :].rearrange("p (t c) -> p t c", c=128),
            stag[:].rearrange("t c -> t () c"))

        # ---------------- main loop (vector engine silent) -------------
        def emit_score(qc, jj):
            pexp_t = ppool.tile([128, 1024], F32, tag="pp", name="pexp")
            qsl = slice(qc * QC, (qc + 1) * QC)
            for h in range(2):
                j = jj + h
                blk = slice(j * 128, (j + 1) * 128)
                reg = pexp_t[:, h * 512:(h + 1) * 512]
                nc.tensor.matmul(reg, kt_f[:, blk], qst_f[:, qsl],
                                 start=True, stop=True)
            return pexp_t

        po = {}
        pexp_cur = emit_score(0, 0)
        for u in range(64):
            qc, jj = u // 32, 2 * (u % 32)
            if jj == 0:
                po[qc] = pacc.tile([D + 1, QC], F32, tag="po", name="po")
            pt = work.tile([128, 1024], BF16, tag="pt", name="pt")
            nc.scalar.activation(pt[:], pexp_cur[:], EXP)
            if u + 1 < 64:
                pexp_cur = emit_score((u + 1) // 32, 2 * ((u + 1) % 32))
            nc.tensor.matmul(po[qc][:], xaug_v[:, jj, :], pt[:, 0:512],
                             start=(jj == 0), stop=False)
            nc.tensor.matmul(po[qc][:], xaug_v[:, jj + 1, :], pt[:, 512:1024],
                             start=False, stop=(jj == 62))

        # ---------------- normalize (tail) ----------------
        o_all = big.tile([128, NQ // 128 * D], F32, name="o_all")
        for qc in range(2):
            ot = work.tile([D + 1, QC], F32, tag="ot", name="ot")
            nc.vector.tensor_copy(ot[:], po[qc][:])
            for h in range(QC // 128):
                ptr_t = ppool.tile([128, 1024], F32, tag="pp", name="ptr")
                ps_t = ptr_t[0:128, 0:D + 1]
                nc.tensor.transpose(ps_t, ot[:, h * 128:(h + 1) * 128],
                                    ident[0:D + 1, 0:D + 1])
                recip = small.tile([128, 1], F32, tag="recip", name="recip")
                nc.vector.reciprocal(recip[:], ps_t[:, D:D + 1])
                t = qc * 4 + h
                nc.vector.tensor_scalar_mul(o_all[:, t * D:(t + 1) * D],
                                            ps_t[:, 0:D], recip[:])
        nc.sync.dma_start(
            out_ap.rearrange("(t p) d -> p t d", p=128),
            o_all[:].rearrange("p (t d) -> p t d", d=D))

    nc.compile()
    return nc


_CACHE = {}


def _get_nc():
    if "nc" not in _CACHE:
        _CACHE["nc"] = build()
    return _CACHE["nc"]


def kernel(x, rotation_params, entangle_params, _trace=False, _nc=None):
    from concourse.bass_utils import run_bass_kernel_spmd
    import ml_dtypes

    bf16 = ml_dtypes.bfloat16
    f32 = np.float32

    x = np.ascontiguousarray(x, dtype=f32)
    rs = np.ascontiguousarray(rotation_params, dtype=f32) / 8.0
    e = np.ascontiguousarray(entangle_params, dtype=f32)
    # permute projection columns so the weakest |E_j|*|R_j| product sits
    # at dim 63, whose fp16 lo-correction is the one dropped
    perm = np.argsort(-(np.linalg.norm(e, axis=0) * np.linalg.norm(rs, axis=0)))
    rs = np.ascontiguousarray(rs[:, perm])
    e = np.ascontiguousarray(e[:, perm])

    xh = x.astype(bf16)
    xl = (x - xh.astype(f32)).astype(bf16)
    xthh = np.ascontiguousarray(np.vstack([xh.T, xh.T]))          # [128, N]
    xtlo = xl.T                                                   # [64, N]
    # paired key-side tiles: [128, 2048] per 1024 cols: hi | lo
    xt = np.zeros((128, 2 * N), dtype=bf16)
    for i in range(8):
        xt[:, i * 2048:i * 2048 + 1024] = xthh[:, i * 1024:(i + 1) * 1024]
        xt[0:D, i * 2048 + 1024:(i + 1) * 2048] = xtlo[:, i * 1024:(i + 1) * 1024]
        xt[D:128, i * 2048 + 1024:(i + 1) * 2048] = xtlo[:, i * 1024:(i + 1) * 1024]

    def hl(w):
        h = w.astype(bf16)
        l = (w - h.astype(f32)).astype(bf16)
        return np.ascontiguousarray(np.vstack([h, l])), h

    rhl, rh = hl(rs)
    ehl, eh = hl(e)

    xaug = np.zeros((N, DP), dtype=bf16)
    xaug[:, :D] = xh
    xaug[:, D] = 1.0
    xaug_p = np.ascontiguousarray(
        xaug.reshape(NKB, 128, DP).transpose(1, 0, 2).reshape(128, NKB * DP))

    ident = np.eye(128, dtype=f32)

    nc = _nc if _nc is not None else _get_nc()
    in_maps = []
    for c in range(NCORES):
        qsl = slice(c * NQ, (c + 1) * NQ)
        wx = np.zeros((128, 2304 + N), dtype=bf16)
        wx[:, 0:64] = rhl
        wx[:, 64:128] = ehl
        wx[0:D, 128:192] = rh
        wx[0:D, 192:256] = eh
        wx[D:128, 128:192] = rh
        wx[D:128, 192:256] = eh
        wx[:, 256:1280] = xthh[:, qsl]
        wx[0:D, 1280:2304] = xtlo[:, qsl]
        wx[D:128, 1280:2304] = xtlo[:, qsl]
        # kt_f's ones row is fp16; store fp16(1.0)'s bit pattern (0x3C00)
        # through the bf16-typed wx buffer
        wx.view(np.uint16)[0:1, 2304:] = 0x3C00
        in_maps.append({
            "wx": wx,
            "xt": xt,
            "ident": ident,
            "xaug": xaug_p,
        })
    res = run_bass_kernel_spmd(nc, in_maps, core_ids=list(range(NCORES)),
                               trace=_trace)
    out = np.concatenate([res.results[c]["out"] for c in range(NCORES)], axis=0)
    if _trace:
        return out, res
    return out
